# revision 1
# baseline (speedup 1.0000x reference)
"""Trainium2 Bass kernel for nn_MultiHeadAttention_67379446939752.

Per-token multi-head attention:
  Q = q @ Wq.T + bq ; K,V likewise        [B,S,D] -> [B,S,H,HD]
  score[t,h,g] = sum_d Q[t,h,d] K[t,g,d]  (per-token HxH gram, no seq mixing)
  attn[t] = softmax(score[t]) @ V[t]      -> [B,S,D]
  out = attn @ Wo.T + bo

Strategy: data-parallel over the 16384 tokens across 8 NeuronCores (2048
tokens/core).  All big matmuls run in float32r (full PE rate at N>=256,
~1e-4 relative error).  Host pre-transposes activations/weights so the
contraction dim lands on SBUF partitions with no on-device transposes.
The per-token 16x16 attention is computed 8 tokens at a time as a single
128x128x128 fp32 matmul whose cross-token blocks are pushed to -1024 in
PSUM by a rank-8 bf16 mask matmul; exp() then zeroes them exactly, so the
block-diagonal softmax needs no masking pass on DVE.
"""
import sys
sys.path.insert(0, "/opt/trn_rl_repo")
import numpy as np
import concourse.bass as bass
import concourse.mybir as mybir
import concourse.bacc as bacc
import concourse.tile as tile
from concourse.bass_utils import run_bass_kernel_spmd

B, S, D, H, HD = 4, 4096, 2048, 16, 128
NCORES = 8
T_FULL = B * S
F32, F32R, BF16 = mybir.dt.float32, mybir.dt.float32r, mybir.dt.bfloat16
KT = D // 128            # contraction tiles
SHIFT = 25.0             # constant softmax shift (softmax-invariant)
NEG = 1024.0             # additive mask magnitude for cross-token blocks
TA = 256                 # token chunk (phase A/B/C share this granularity)
Exp = mybir.ActivationFunctionType.Exp


def mask_consts():
    # u8[r,(t,h)] = 1 if t==r ; v8[r,(t',g)] = -NEG*(1 - (t'==r))
    u = np.zeros((8, 128), np.float32)
    for r in range(8):
        u[r, r * 16:(r + 1) * 16] = 1.0
    v = np.full((8, 128), -NEG, np.float32)
    for r in range(8):
        v[r, r * 16:(r + 1) * 16] = 0.0
    return u, v


def build(T, debug=False, repeat=1, trace_sim=False):
    import ml_dtypes
    TAe = min(TA, T)
    NCH = T // TAe           # chunks
    NBK = TAe // 8           # 8-token blocks per chunk
    nc = bacc.Bacc(None, target_bir_lowering=False)
    dt_in = lambda n, s: nc.dram_tensor(n, s, F32R, kind="ExternalInput")
    qT = dt_in("qT", [D, T]); kT = dt_in("kT", [D, T]); vT = dt_in("vT", [D, T])
    WqT = dt_in("WqT", [D, D]); WkT = dt_in("WkT", [D, D])
    WvT = dt_in("WvT", [D, D]); WoT = dt_in("WoT", [D, D])
    bqT = nc.dram_tensor("bqT", [128, H], F32, kind="ExternalInput")
    bkT = nc.dram_tensor("bkT", [128, H], F32, kind="ExternalInput")
    bvT = nc.dram_tensor("bvT", [128, H], F32, kind="ExternalInput")
    bo_row = nc.dram_tensor("bo_row", [1, D], F32R, kind="ExternalInput")
    ones_row = nc.dram_tensor("ones_row", [1, 128], F32R, kind="ExternalInput")
    out_d = nc.dram_tensor("out", [T, D], F32, kind="ExternalOutput")
    dbg = {}
    if debug:
        for n, shp in (("dQT", [128, T * H]), ("dKT", [128, T * H]),
                       ("dV", [128, T * H]), ("dATT", [D, T])):
            dbg[n] = nc.dram_tensor(n, shp, F32, kind="ExternalOutput")

    u8_np, v8_np = mask_consts()
    u8_d = nc.inline_tensor(u8_np.astype(ml_dtypes.bfloat16), "u8c")
    v8_d = nc.inline_tensor(v8_np.astype(ml_dtypes.bfloat16), "v8c")
    id_d = nc.inline_tensor(np.eye(128, dtype=np.float32), "id128").bitcast(F32R)

    with tile.TileContext(nc, trace_sim=trace_sim) as tc:
        with (
            tc.tile_pool(name="dram", bufs=1, space="DRAM") as dpool,
            tc.tile_pool(name="const", bufs=1) as cpool,
        ):
            # per-chunk spill tiles (fine-grained cross-phase deps)
            QT_ds = [dpool.tile([128, TAe * H], F32R, tag=f"QTd{i}", name=f"QTd{i}") for i in range(NCH)]
            KT_ds = [dpool.tile([128, TAe * H], F32R, tag=f"KTd{i}", name=f"KTd{i}") for i in range(NCH)]
            VT_ds = [dpool.tile([128, TAe * H], F32R, tag=f"VTd{i}", name=f"VTd{i}") for i in range(NCH)]
            ATT_ds = [dpool.tile([D, TAe], F32R, tag=f"ATTd{i}", name=f"ATTd{i}") for i in range(NCH)]

            u8 = cpool.tile([8, 128], BF16, tag="u8")
            v8 = cpool.tile([8, 128], BF16, tag="v8")
            ident = cpool.tile([128, 128], F32R, tag="ident")
            nc.sync.dma_start(u8[:], u8_d[:])
            nc.sync.dma_start(v8[:], v8_d[:])
            nc.sync.dma_start(ident[:], id_d[:])
            biasq = cpool.tile([128, H], F32, tag="bq")
            biask = cpool.tile([128, H], F32, tag="bk")
            biasv = cpool.tile([128, H], F32, tag="bvt")
            bor = cpool.tile([1, D], F32R, tag="bo")
            onesr = cpool.tile([1, 128], F32R, tag="ones")
            nc.sync.dma_start(biasq[:], bqT[:])
            nc.sync.dma_start(biask[:], bkT[:])
            nc.sync.dma_start(biasv[:], bvT[:])
            nc.sync.dma_start(bor[:], bo_row[:])
            nc.sync.dma_start(onesr[:], ones_row[:])
            shiftc = cpool.tile([128, 1], F32, tag="shiftc")
            nc.vector.memset(shiftc[:], -SHIFT)

            def _load_w(pool, win, tag):
                # separate quarter tiles -> first matmuls start after 1/4 load
                src = win.ap().rearrange("(it p) j -> p it j", p=128)
                parts = []
                for q in range(4):
                    wq = pool.tile([128, 4, D], F32R, tag=f"{tag}{q}", name=f"{tag}{q}")
                    nc.sync.dma_start(wq[:], src[:, q * 4:(q + 1) * 4, :])
                    parts.append(wq)
                return parts

            def _phases():
                # ---------------- Phase A: projections ----------------
                with (
                    tc.tile_pool(name="wt", bufs=1) as wpool,
                    tc.tile_pool(name="xs", bufs=2) as xpool,
                    tc.tile_pool(name="psA", bufs=8, space="PSUM") as psA,
                    tc.tile_pool(name="stA", bufs=1) as stA,
                ):
                    for xin, win, bias, spills in (
                        (qT, WqT, biasq, QT_ds),
                        (kT, WkT, biask, KT_ds),
                        (vT, WvT, biasv, VT_ds),
                    ):
                        xs0 = xpool.tile([128, KT, TAe], F32R, tag="xs", name="xs0")
                        nc.sync.dma_start(
                            xs0[:], xin[:, 0:TAe].rearrange("(it p) t -> p it t", p=128))
                        wt = _load_w(wpool, win, "wt")
                        for c in range(NCH):
                            if c == 0:
                                xs = xs0
                            else:
                                xs = xpool.tile([128, KT, TAe], F32R, tag="xs")
                                nc.sync.dma_start(
                                    xs[:], xin[:, c * TAe:(c + 1) * TAe].rearrange(
                                        "(it p) t -> p it t", p=128))
                            stg = stA.tile([128, TAe, H], F32R, tag="stA")
                            for jh in range(2):
                                pss = [psA.tile([128, TAe], F32, tag="psA",
                                                name=f"psA{jh}_{j}") for j in range(8)]
                                for q in range(4):
                                    for jl in range(8):
                                        jt = jh * 8 + jl
                                        for kl in range(4):
                                            ki = q * 4 + kl
                                            nc.tensor.matmul(
                                                pss[jl][:], wt[q][:, kl, jt * 128:(jt + 1) * 128],
                                                xs[:, ki, :], start=(ki == 0), stop=(ki == KT - 1))
                                for jl in range(8):
                                    jt = jh * 8 + jl
                                    nc.any.tensor_scalar_add(stg[:, :, jt], pss[jl][:],
                                                             bias[:, jt:jt + 1])
                            nc.sync.dma_start(
                                spills[c][:], stg[:].rearrange("p t h -> p (t h)"))

                # ---------------- Phase B (with Wo q0 prefetch) + C ----------------
                with tc.tile_pool(name="wo", bufs=1) as wopool:
                  wsrc = WoT.ap().rearrange("(h p) j -> p h j", p=128)
                  wo0 = wopool.tile([128, 4, D], F32R, tag="wo0", name="wo0")
                  nc.sync.dma_start(wo0[:], wsrc[:, 0:4, :])
                  with (
                      tc.tile_pool(name="qk", bufs=2) as qkpool,
                      tc.tile_pool(name="vb", bufs=2) as vpool,
                      tc.tile_pool(name="attc", bufs=2) as apool,
                      tc.tile_pool(name="eb", bufs=6) as epool,
                      tc.tile_pool(name="zb", bufs=8) as zpool,
                      tc.tile_pool(name="psS", bufs=2, space="PSUM") as psS,
                      tc.tile_pool(name="psT", bufs=2, space="PSUM") as psT,
                      tc.tile_pool(name="psV", bufs=2, space="PSUM") as psV,
                      tc.tile_pool(name="psA2", bufs=2, space="PSUM") as psA2,
                  ):
                      for c in range(NCH):
                          t0c = c * TAe
                          QTs = qkpool.tile([128, TAe, H], F32R, tag="QTs")
                          KTs = qkpool.tile([128, TAe, H], F32R, tag="KTs")
                          nc.gpsimd.dma_start(
                              QTs[:], QT_ds[c][:].rearrange("p (t h) -> p t h", h=H))
                          nc.gpsimd.dma_start(
                              KTs[:], KT_ds[c][:].rearrange("p (t h) -> p t h", h=H))
                          VTs = vpool.tile([128, TAe, H], F32R, tag="VTs")
                          nc.sync.dma_start(
                              VTs[:], VT_ds[c][:].rearrange("p (t h) -> p t h", h=H))
                          ATTc = apool.tile([128, H, TAe], F32R, tag="ATTc")
                          for bk in range(NBK):
                              sl = slice(bk * 8, (bk + 1) * 8)
                              w0 = (bk // 2) * 2            # even-aligned 2-block window
                              off = (bk % 2) * 128          # valid column offset
                              slw = slice(w0 * 8, (w0 + 2) * 8)
                              ps_b = psS.tile([128, 256], F32, tag="ps_s")
                              nc.tensor.matmul(
                                  ps_b[:],
                                  QTs[:, sl, :].rearrange("p t h -> p (t h)"),
                                  KTs[:, slw, :].rearrange("p t h -> p (t h)"),
                                  start=True, stop=False, skip_group_check=True)
                              nc.tensor.matmul(ps_b[:, off:off + 128], u8[:], v8[:],
                                               start=False, stop=True, skip_group_check=True)
                              E = epool.tile([128, 128], F32, tag="E")
                              Z = zpool.tile([128, 1], F32, tag="Z")
                              nc.scalar.activation(E[:], ps_b[:, off:off + 128], Exp,
                                                   bias=shiftc[:], accum_out=Z[:])
                              R = zpool.tile([128, 1], F32, tag="R")
                              nc.vector.reciprocal(R[:], Z[:])
                              Wb = epool.tile([128, 128], F32R, tag="Wb")
                              nc.vector.tensor_scalar_mul(Wb[:], E[:], R[:])
                              ps_t = psT.tile([128, 128], F32R, tag="ps_t")
                              nc.tensor.transpose(ps_t[:], Wb[:], ident[:])
                              WTs = epool.tile([128, 128], F32R, tag="WTs")
                              nc.any.tensor_copy(WTs[:], ps_t[:])
                              ps_v = psV.tile([128, 128], F32R, tag="ps_v")
                              nc.tensor.transpose(
                                  ps_v[:], VTs[:, sl, :].rearrange("p t h -> p (t h)"),
                                  ident[:])
                              Vb = epool.tile([128, 128], F32R, tag="Vb")
                              nc.any.tensor_copy(Vb[:], ps_v[:])
                              ps_a = psA2.tile([128, 128], F32, tag="ps_a")
                              nc.tensor.matmul(ps_a[:], Vb[:], WTs[:],
                                               start=True, stop=True)
                              nc.any.tensor_copy(
                                  ATTc[:, :, bk * 8:(bk + 1) * 8].rearrange("p h t -> p t h"),
                                  ps_a[:].rearrange("p (t h) -> p t h", t=8))
                          nc.sync.dma_start(
                              ATT_ds[c][:].rearrange("(h p) t -> p h t", p=128), ATTc[:])

                  # ---------------- Phase C: output projection ----------------
                  with (
                      tc.tile_pool(name="wo", bufs=1) as wopool,
                      tc.tile_pool(name="ca", bufs=2) as capool,
                      tc.tile_pool(name="psC", bufs=8, space="PSUM") as psC,
                      tc.tile_pool(name="stC", bufs=4) as stC,
                  ):
                      ATTs0 = capool.tile([128, H, TAe], F32R, tag="ATTs", name="ATTs0")
                      nc.sync.dma_start(
                          ATTs0[:], ATT_ds[0][:].rearrange("(h p) t -> p h t", p=128))
                      wo = [wo0]
                      for q in range(1, 4):
                          wq = wopool.tile([128, 4, D], F32R, tag=f"wo{q}", name=f"wo{q}")
                          nc.sync.dma_start(wq[:], wsrc[:, q * 4:(q + 1) * 4, :])
                          wo.append(wq)
                      for cc in range(NCH):
                          if cc == 0:
                              ATTs = ATTs0
                          else:
                              ATTs = capool.tile([128, H, TAe], F32R, tag="ATTs")
                              nc.sync.dma_start(
                                  ATTs[:], ATT_ds[cc][:].rearrange("(h p) t -> p h t", p=128))
                          tjs = [(tt, jc) for tt in range(TAe // 128) for jc in range(D // 512)]
                          pss = [psC.tile([128, 512], F32, tag="psC", name=f"psC{i}")
                                 for i in range(len(tjs))]
                          for hq in range(4):
                              for i, (tt, jc) in enumerate(tjs):
                                  for hl in range(4):
                                      h = hq * 4 + hl
                                      nc.tensor.matmul(
                                          pss[i][:], ATTs[:, h, tt * 128:(tt + 1) * 128],
                                          wo[hq][:, hl, jc * 512:(jc + 1) * 512],
                                          start=(h == 0), stop=False)
                          for i, (tt, jc) in enumerate(tjs):
                              nc.tensor.matmul(pss[i][:], onesr[:], bor[:, jc * 512:(jc + 1) * 512],
                                               start=False, stop=True)
                              st = stC.tile([128, 512], F32, tag="stC")
                              nc.any.tensor_copy(st[:], pss[i][:])
                              nc.sync.dma_start(
                                  out_d[cc * TAe + tt * 128: cc * TAe + (tt + 1) * 128,
                                        jc * 512:(jc + 1) * 512], st[:])

            for _rep in range(repeat):
                _phases()

            if debug:
                with tc.tile_pool(name="dbgp", bufs=2) as dbgp:
                    def dump(name, srcs, width):
                        for i, srct in enumerate(srcs):
                            flat = srct[:].rearrange("p a b -> p (a b)") \
                                if len(srct.shape) == 3 else srct[:]
                            rows = flat.shape[0]
                            for r0 in range(0, rows, 128):
                                tcp = dbgp.tile([128, width], F32, tag="dbg")
                                nc.sync.dma_start(tcp[:], flat[r0:r0 + 128, :].bitcast(F32))
                                nc.sync.dma_start(
                                    dbg[name][r0:r0 + 128, i * width:(i + 1) * width], tcp[:])
                    dump("dQT", QT_ds, TAe * H)
                    dump("dKT", KT_ds, TAe * H)
                    dump("dV", VT_ds, TAe * H)
                    dump("dATT", ATT_ds, TAe)
    nc.compile()
    return nc


_cache = {}


def get_nc(T):
    if T not in _cache:
        _cache[T] = build(T)
    return _cache[T]


def make_in_maps(q, k, v, Wq, bq, Wk, bk, Wv, bv, Wo, bo, ncores=NCORES, T=None):
    f = np.float32
    q = np.asarray(q, f).reshape(-1, D)
    k = np.asarray(k, f).reshape(-1, D)
    v = np.asarray(v, f).reshape(-1, D)
    if T is None:
        T = q.shape[0] // ncores
    WqT = np.ascontiguousarray(np.asarray(Wq, f).T)
    WkT = np.ascontiguousarray(np.asarray(Wk, f).T)
    WvT = np.ascontiguousarray(np.asarray(Wv, f).T)
    WoT = np.ascontiguousarray(np.asarray(Wo, f).T)
    bqT = np.ascontiguousarray(np.asarray(bq, f).reshape(H, 128).T)
    bkT = np.ascontiguousarray(np.asarray(bk, f).reshape(H, 128).T)
    bvTc = np.ascontiguousarray(np.asarray(bv, f).reshape(H, 128).T)
    bor = np.asarray(bo, f).reshape(1, D)
    maps = []
    for c in range(ncores):
        sl = slice(c * T, (c + 1) * T)
        maps.append({
            "qT": np.ascontiguousarray(q[sl].T),
            "kT": np.ascontiguousarray(k[sl].T),
            "vT": np.ascontiguousarray(v[sl].T),
            "WqT": WqT, "WkT": WkT, "WvT": WvT, "WoT": WoT,
            "bqT": bqT, "bkT": bkT, "bvT": bvTc, "bo_row": bor,
            "ones_row": np.ones((1, 128), f),
        })
    return maps, T


def kernel(q, k, v, Wq, bq, Wk, bk, Wv, bv, Wo, bo):
    maps, T = make_in_maps(q, k, v, Wq, bq, Wk, bk, Wv, bv, Wo, bo)
    nc = get_nc(T)
    res = run_bass_kernel_spmd(nc, maps, list(range(NCORES)))
    out = np.concatenate([np.asarray(r["out"]) for r in res.results], axis=0)
    return out.reshape(B, S, D).astype(np.float32)



# revision 22
# speedup vs baseline: 42.8223x; 42.8223x over previous
"""Trainium2 Bass kernel for nn_MultiHeadAttention_67379446939752.

Per-token multi-head attention:
  Q = q @ Wq.T + bq ; K,V likewise        [B,S,D] -> [B,S,H,HD]
  score[t,h,g] = sum_d Q[t,h,d] K[t,g,d]  (per-token HxH gram, no seq mixing)
  attn[t] = softmax(score[t]) @ V[t]      -> [B,S,D]
  out = attn @ Wo.T + bo
  B,S,D,H,HD = 4,4096,2048,16,128

Strategy: data-parallel over the 16384 tokens across 8 NeuronCores (2048
tokens/core).  The Q/K score path runs in float32r end-to-end (softmax
exponentiates score errors, so bf16 anywhere on that path blows the error
budget); the value path (V, softmax weights, Wo, output projection) is bf16.

Phase A streams weight sets so loads hide under compute: q is one full pass
with eight 2MB weight tiles (release of tile e staggers, so the k pass's
first set starts loading ~12us before q ends); k and v are two half-passes
each (one 8MB set per h-half) alternating between the same tile slots, so
every later set loads entirely under the previous pass.  Q spills h-
interleaved [d, t, 16h]; K/V spill per-half [d, t, 8h] (all contiguous).

Phase B consumes Q as matmul lhsT directly (rows in (t,h) order), K in
half-layout as the 3-free-dim moving window (score columns in (jh,t,g')
order), and V via two per-half PE transposes with partition-offset PSUM
writes, which lands Vb rows in the same (jh,t,g') order as the score
columns.  The per-token 16x16 softmax uses a constant shift (softmax-
invariant) and a rank-8 bf16 mask matmul that pushes cross-token entries to
-1024 so exp() zeroes them exactly.  Phase C (output projection) shares the
pool scope with B and consumes B's per-chunk ATT tiles straight from SBUF,
so its matmuls interleave with B on the PE and no ATT spill is needed.
GPSIMD/Pool never touches PSUM (hardware restriction); all PSUM drains go
through DVE/Act, spread to keep every engine under the PE's critical path.
"""
import sys
sys.path.insert(0, "/opt/trn_rl_repo")
import numpy as np
import concourse.bass as bass
import concourse.mybir as mybir
import concourse.bacc as bacc
import concourse.tile as tile
from concourse.bass_utils import run_bass_kernel_spmd

B, S, D, H, HD = 4, 4096, 2048, 16, 128
NCORES = 8
T_FULL = B * S
F32, F32R, BF16 = mybir.dt.float32, mybir.dt.float32r, mybir.dt.bfloat16
KT = D // 128            # contraction tiles
SHIFT = 25.0             # constant softmax shift (softmax-invariant)
NEG = 1024.0             # additive mask magnitude for cross-token blocks
TA = 256                 # token chunk (phases share this granularity)
Exp = mybir.ActivationFunctionType.Exp
Copy = mybir.ActivationFunctionType.Copy


def mask_consts():
    # Score rows i are (t,h) t-major: row token = i//16.
    # u8[r,i] = 1 iff i//16 == r.
    u = np.zeros((8, 128), np.float32)
    for r in range(8):
        u[r, r * 16:(r + 1) * 16] = 1.0
    # Score cols within one h-half are (t_local, g') with col token = x//8.
    # v8s[r,x] = -NEG*(1 - (x//8 == r)); the same 64-col pattern serves both
    # j-halves and both blocks of a window pair.
    tokc = np.arange(64) // 8
    v = -NEG * (1.0 - (tokc[None, :] == np.arange(8)[:, None])).astype(np.float32)
    return u, v


def build(T, repeat=1, trace_sim=False):
    import ml_dtypes
    TAe = min(TA, T)
    NCH = T // TAe           # chunks
    NBK = TAe // 8           # 8-token blocks per chunk
    nc = bacc.Bacc(None, target_bir_lowering=False)
    dt_in = lambda n, s: nc.dram_tensor(n, s, F32R, kind="ExternalInput")
    qT = dt_in("qT", [D, T]); kT = dt_in("kT", [D, T]); vT = dt_in("vT", [D, T])
    WqT = dt_in("WqT", [D, D]); WkT = dt_in("WkT", [D, D])
    WvT = dt_in("WvT", [D, D])
    WoT = nc.dram_tensor("WoT", [D, D], BF16, kind="ExternalInput")
    bqT = nc.dram_tensor("bqT", [128, H], F32, kind="ExternalInput")
    bkT = nc.dram_tensor("bkT", [128, H], F32, kind="ExternalInput")
    bvT = nc.dram_tensor("bvT", [128, H], F32, kind="ExternalInput")
    bo_bc = nc.dram_tensor("bo_bc", [128, D], F32, kind="ExternalInput")
    out_d = nc.dram_tensor("out", [T, D], F32, kind="ExternalOutput")

    u8_np, v8s_np = mask_consts()
    u8_d = nc.inline_tensor(u8_np.astype(ml_dtypes.bfloat16), "u8c")
    v8_d = nc.inline_tensor(v8s_np.astype(ml_dtypes.bfloat16), "v8c")
    idb_d = nc.inline_tensor(np.eye(128, dtype=ml_dtypes.bfloat16), "idb128")

    with tile.TileContext(nc, trace_sim=trace_sim) as tc:
        with (
            tc.tile_pool(name="dram", bufs=1, space="DRAM") as dpool,
            tc.tile_pool(name="const", bufs=1) as cpool,
        ):
            # Q spill: h-interleaved per chunk; K/V spill per (h-half, chunk)
            QT_ds = [dpool.tile([128, TAe * H], F32R, tag=f"QTd{i}", name=f"QTd{i}")
                     for i in range(NCH)]
            KT_ds = [[dpool.tile([128, TAe * 8], F32R, tag=f"KTd{j}_{i}", name=f"KTd{j}_{i}")
                      for i in range(NCH)] for j in range(2)]
            VT_ds = [[dpool.tile([128, TAe * 8], BF16, tag=f"VTd{j}_{i}", name=f"VTd{j}_{i}")
                      for i in range(NCH)] for j in range(2)]

            u8 = cpool.tile([8, 128], BF16, tag="u8")
            v8s = cpool.tile([8, 64], BF16, tag="v8s")
            identb = cpool.tile([128, 128], BF16, tag="identb")
            nc.sync.dma_start(u8[:], u8_d[:])
            nc.sync.dma_start(v8s[:], v8_d[:])
            nc.sync.dma_start(identb[:], idb_d[:])
            biasq = cpool.tile([128, H], F32, tag="bq")
            biask = cpool.tile([128, H], F32, tag="bk")
            biasv = cpool.tile([128, H], F32, tag="bvt")
            nc.sync.dma_start(biasq[:], bqT[:])
            nc.sync.dma_start(biask[:], bkT[:])
            nc.sync.dma_start(biasv[:], bvT[:])
            shiftc = cpool.tile([128, 1], F32, tag="shiftc")
            nc.vector.memset(shiftc[:], -SHIFT)

            def _phases():
                # ---------------- Phase A: projections ----------------
                with (
                    tc.tile_pool(name="wt", bufs=1) as wpool,
                    tc.tile_pool(name="xs", bufs=2) as xpool,
                    tc.tile_pool(name="psA", bufs=8, space="PSUM") as psA,
                    tc.tile_pool(name="stA", bufs=2) as stA,
                ):
                    def drain(stg, pss, bias, jh_base):
                        for jl in range(8):
                            if jl % 2 == 0:
                                nc.vector.tensor_scalar_add(
                                    stg[:, :, jl], pss[jl][:],
                                    bias[:, jh_base + jl:jh_base + jl + 1])
                            else:
                                nc.scalar.add(stg[:, :, jl], pss[jl][:],
                                              bias[:, jh_base + jl:jh_base + jl + 1])

                    # --- q: one full pass, eight 2MB weight tiles ---
                    wsrc = WqT.ap().rearrange("(it p) j -> p it j", p=128)
                    wq = []
                    for e in range(8):
                        we = wpool.tile([128, 2, D], F32R, tag=f"w{e}", name=f"wq{e}")
                        qeng = nc.sync if e % 2 == 0 else nc.gpsimd
                        qeng.dma_start(we[:], wsrc[:, e * 2:(e + 1) * 2, :])
                        wq.append(we)
                    for c in range(NCH):
                        xs = xpool.tile([128, KT, TAe], F32R, tag="xs", name=f"xsq{c}")
                        nc.scalar.dma_start(
                            xs[:], qT[:, c * TAe:(c + 1) * TAe].rearrange(
                                "(it p) t -> p it t", p=128))
                        stg = stA.tile([128, TAe, H], F32R, tag="stA", name=f"stq{c}")
                        for jh in range(2):
                            pss = [psA.tile([128, TAe], F32, tag="psA",
                                            name=f"psq{c}_{jh}_{j}") for j in range(8)]
                            for e in range(8):
                                for jl in range(8):
                                    jt = jh * 8 + jl
                                    for kl in range(2):
                                        ki = e * 2 + kl
                                        nc.tensor.matmul(
                                            pss[jl][:], wq[e][:, kl, jt * 128:(jt + 1) * 128],
                                            xs[:, ki, :], start=(ki == 0), stop=(ki == KT - 1))
                            drain(stg[:, :, jh * 8:(jh + 1) * 8], pss, biasq, jh * 8)
                        nc.sync.dma_start(
                            QT_ds[c][:], stg[:].rearrange("p t h -> p (t h)"))

                    # --- k, v: two half-passes each, alternating slot groups ---
                    for xin, win, bias, spills, sdt in (
                        (kT, WkT, biask, KT_ds, F32R),
                        (vT, WvT, biasv, VT_ds, BF16),
                    ):
                        wsrcH = win.ap().rearrange("(it p) j -> p it j", p=128)
                        for jh in range(2):
                            wh = []
                            for q in range(4):
                                wt_ = wpool.tile([128, 4, D // 2], F32R,
                                                 tag=f"w{jh * 4 + q}",
                                                 name=f"wh{jh}_{q}")
                                nc.sync.dma_start(
                                    wt_[:], wsrcH[:, q * 4:(q + 1) * 4,
                                                  jh * (D // 2):(jh + 1) * (D // 2)])
                                wh.append(wt_)
                            for c in range(NCH):
                                xs = xpool.tile([128, KT, TAe], F32R, tag="xs")
                                nc.scalar.dma_start(
                                    xs[:], xin[:, c * TAe:(c + 1) * TAe].rearrange(
                                        "(it p) t -> p it t", p=128))
                                stg = stA.tile([128, TAe, 8], sdt, tag="stA",
                                               name=f"sth{jh}_{c}")
                                pss = [psA.tile([128, TAe], F32, tag="psA",
                                                name=f"psh{jh}_{c}_{j}") for j in range(8)]
                                for q in range(4):
                                    for jl in range(8):
                                        for kl in range(4):
                                            ki = q * 4 + kl
                                            nc.tensor.matmul(
                                                pss[jl][:], wh[q][:, kl, jl * 128:(jl + 1) * 128],
                                                xs[:, ki, :], start=(ki == 0), stop=(ki == KT - 1))
                                drain(stg, pss, bias, jh * 8)
                                nc.sync.dma_start(
                                    spills[jh][c][:], stg[:].rearrange("p t h -> p (t h)"))

                # -------- Phases B + C: attention + output projection --------
                with tc.tile_pool(name="wo", bufs=1) as wopool:
                  wsrcO = WoT.ap().rearrange("(h p) j -> p h j", p=128)
                  wo = []
                  for q in range(4):
                      wq_ = wopool.tile([128, 4, D], BF16, tag=f"wo{q}", name=f"wo{q}")
                      nc.scalar.dma_start(wq_[:], wsrcO[:, q * 4:(q + 1) * 4, :])
                      wo.append(wq_)
                  bias_bc = wopool.tile([128, D], F32, tag="bobc")
                  nc.scalar.dma_start(bias_bc[:], bo_bc[:])
                  with (
                      tc.tile_pool(name="qk", bufs=2) as qkpool,
                      tc.tile_pool(name="vb", bufs=2) as vpool,
                      tc.tile_pool(name="attc", bufs=2) as apool,
                      tc.tile_pool(name="eb", bufs=4) as epool,
                      tc.tile_pool(name="zb", bufs=6) as zpool,
                      tc.tile_pool(name="stC", bufs=2) as stC,
                      tc.tile_pool(name="psS", bufs=2, space="PSUM") as psS,
                      tc.tile_pool(name="psTV", bufs=2, space="PSUM") as psTV,
                      tc.tile_pool(name="psA2", bufs=2, space="PSUM") as psA2,
                      tc.tile_pool(name="psC", bufs=2, space="PSUM") as psC,
                  ):
                      attcs = []
                      for c in range(NCH):
                          QTs = qkpool.tile([128, TAe, H], F32R, tag="QTs")
                          nc.sync.dma_start(
                              QTs[:], QT_ds[c][:].rearrange("p (t h) -> p t h", h=H))
                          KTs = qkpool.tile([128, 2, TAe, 8], F32R, tag="KTs")
                          VTs = vpool.tile([128, 2, TAe, 8], BF16, tag="VTs")
                          for j in range(2):
                              nc.gpsimd.dma_start(
                                  KTs[:, j], KT_ds[j][c][:].rearrange("p (t h) -> p t h", h=8))
                              nc.sync.dma_start(
                                  VTs[:, j], VT_ds[j][c][:].rearrange("p (t h) -> p t h", h=8))
                          ATTc = apool.tile([128, H, TAe], BF16, tag="ATTc")
                          attcs.append(ATTc)
                          for pr in range(NBK // 2):
                              ba, bb = 2 * pr, 2 * pr + 1
                              slw = slice(ba * 8, (ba + 2) * 8)
                              kw = KTs[:, :, slw, :]
                              ps_s = psS.tile([128, 512], F32, tag="ps_s2")
                              for blk, base, vo in ((ba, 0, 0), (bb, 256, 64)):
                                  sl = slice(blk * 8, (blk + 1) * 8)
                                  nc.tensor.matmul(
                                      ps_s[:, base:base + 256],
                                      QTs[:, sl, :].rearrange("p t h -> p (t h)"),
                                      kw, start=True, stop=False, skip_group_check=True)
                                  for j in range(2):
                                      nc.tensor.matmul(
                                          ps_s[:, base + j * 128 + vo:
                                               base + j * 128 + vo + 64],
                                          u8[:], v8s[:], start=False, stop=(j == 1),
                                          skip_group_check=True)
                              Z2 = zpool.tile([128, 2], F32, tag="Z2")
                              E2 = epool.tile([128, 2, 2, 64], BF16, tag="E2")
                              sega = ps_s[:, 0:256].rearrange("p (j x) -> p j x", j=2)
                              segb = ps_s[:, 256:512].rearrange("p (j x) -> p j x", j=2)
                              nc.scalar.activation(E2[:, 0], sega[:, :, 0:64], Exp,
                                                   bias=shiftc[:], accum_out=Z2[:, 0:1])
                              nc.scalar.activation(E2[:, 1], segb[:, :, 64:128], Exp,
                                                   bias=shiftc[:], accum_out=Z2[:, 1:2])
                              R2 = zpool.tile([128, 2], F32, tag="R2")
                              nc.vector.reciprocal(R2[:], Z2[:])
                              Wb2 = epool.tile([128, 2, 128], BF16, tag="Wb2")
                              nc.gpsimd.tensor_scalar_mul(
                                  Wb2[:, 0, :], E2[:, 0].rearrange("p j x -> p (j x)"),
                                  R2[:, 0:1])
                              nc.gpsimd.tensor_scalar_mul(
                                  Wb2[:, 1, :], E2[:, 1].rearrange("p j x -> p (j x)"),
                                  R2[:, 1:2])
                              ps_tv = psTV.tile([128, 512], BF16, tag="ps_tv")
                              nc.tensor.transpose(ps_tv[:, 0:128], Wb2[:, 0, :], identb[:])
                              nc.tensor.transpose(ps_tv[:, 128:256], Wb2[:, 1, :], identb[:])
                              for blk, base in ((ba, 256), (bb, 384)):
                                  slb = slice(blk * 8, (blk + 1) * 8)
                                  for j in range(2):
                                      nc.tensor.transpose(
                                          ps_tv[j * 64:(j + 1) * 64, base:base + 128],
                                          VTs[:, j, slb, :].rearrange("p t h -> p (t h)"),
                                          identb[:])
                              TV2 = epool.tile([128, 512], BF16, tag="TV2")
                              nc.vector.tensor_copy(TV2[:], ps_tv[:])
                              ps_a = psA2.tile([128, 256], F32, tag="ps_a")
                              nc.tensor.matmul(ps_a[:, 0:128], TV2[:, 256:384],
                                               TV2[:, 0:128], start=True, stop=True)
                              nc.tensor.matmul(ps_a[:, 128:256], TV2[:, 384:512],
                                               TV2[:, 128:256], start=True, stop=True)
                              dst = ATTc[:, :, pr * 16:(pr + 1) * 16].rearrange(
                                  "p h (bk tl) -> p bk tl h", bk=2)
                              src = ps_a[:].rearrange("p (bk tl h) -> p bk tl h",
                                                      bk=2, tl=8)
                              if pr % 2 == 0:
                                  nc.scalar.activation(dst, src, Copy)
                              else:
                                  nc.vector.tensor_copy(dst, src)

                      # ---- Phase C: output projection, interleaved with B ----
                      for cc in range(NCH):
                          ATTs = attcs[cc]
                          for tt in range(TAe // 128):
                              for jca in range(4):
                                  ps = psC.tile([128, 512], F32, tag="psC",
                                                name=f"psC{cc}_{tt}_{jca}")
                                  for hq in range(4):
                                      for hl in range(4):
                                          h = hq * 4 + hl
                                          nc.tensor.matmul(
                                              ps[:], ATTs[:, h, tt * 128:(tt + 1) * 128],
                                              wo[hq][:, hl, jca * 512:(jca + 1) * 512],
                                              start=(h == 0), stop=(h == 15))
                                  st = stC.tile([128, 512], F32, tag="stC")
                                  nc.vector.tensor_add(st[:], ps[:],
                                                       bias_bc[:, jca * 512:(jca + 1) * 512])
                                  nc.sync.dma_start(
                                      out_d[cc * TAe + tt * 128: cc * TAe + (tt + 1) * 128,
                                            jca * 512:(jca + 1) * 512], st[:])

            for _rep in range(repeat):
                _phases()
    nc.compile()
    return nc


_cache = {}


def get_nc(T, repeat=1):
    key = (T, repeat)
    if key not in _cache:
        _cache[key] = build(T, repeat=repeat)
    return _cache[key]


def make_in_maps(q, k, v, Wq, bq, Wk, bk, Wv, bv, Wo, bo, ncores=NCORES, T=None):
    import ml_dtypes
    f = np.float32
    q = np.asarray(q, f).reshape(-1, D)
    k = np.asarray(k, f).reshape(-1, D)
    v = np.asarray(v, f).reshape(-1, D)
    if T is None:
        T = q.shape[0] // ncores
    WqT = np.ascontiguousarray(np.asarray(Wq, f).T)
    WkT = np.ascontiguousarray(np.asarray(Wk, f).T)
    WvT = np.ascontiguousarray(np.asarray(Wv, f).T)
    WoT = np.ascontiguousarray(np.asarray(Wo, f).T.astype(ml_dtypes.bfloat16))
    bqT = np.ascontiguousarray(np.asarray(bq, f).reshape(H, 128).T)
    bkT = np.ascontiguousarray(np.asarray(bk, f).reshape(H, 128).T)
    bvTc = np.ascontiguousarray(np.asarray(bv, f).reshape(H, 128).T)
    bo_bc = np.ascontiguousarray(
        np.broadcast_to(np.asarray(bo, f).reshape(1, D), (128, D)))
    maps = []
    for c in range(ncores):
        sl = slice(c * T, (c + 1) * T)
        maps.append({
            "qT": np.ascontiguousarray(q[sl].T),
            "kT": np.ascontiguousarray(k[sl].T),
            "vT": np.ascontiguousarray(v[sl].T),
            "WqT": WqT, "WkT": WkT, "WvT": WvT, "WoT": WoT,
            "bqT": bqT, "bkT": bkT, "bvT": bvTc, "bo_bc": bo_bc,
        })
    return maps, T


def kernel(q, k, v, Wq, bq, Wk, bk, Wv, bv, Wo, bo):
    maps, T = make_in_maps(q, k, v, Wq, bq, Wk, bk, Wv, bv, Wo, bo)
    nc = get_nc(T)
    res = run_bass_kernel_spmd(nc, maps, list(range(NCORES)))
    out = np.concatenate([np.asarray(r["out"]) for r in res.results], axis=0)
    return out.reshape(B, S, D).astype(np.float32)


# revision 23
# speedup vs baseline: 47.7655x; 1.1154x over previous
"""Trainium2 Bass kernel for nn_MultiHeadAttention_67379446939752.

Per-token multi-head attention:
  Q = q @ Wq.T + bq ; K,V likewise        [B,S,D] -> [B,S,H,HD]
  score[t,h,g] = sum_d Q[t,h,d] K[t,g,d]  (per-token HxH gram, no seq mixing)
  attn[t] = softmax(score[t]) @ V[t]      -> [B,S,D]
  out = attn @ Wo.T + bo

Strategy: data-parallel over the 16384 tokens across 8 NeuronCores (2048
tokens/core).  All big matmuls run in float32r (full PE rate at N>=256,
~1e-4 relative error).  Host pre-transposes activations/weights so the
contraction dim lands on SBUF partitions with no on-device transposes.
The per-token 16x16 attention is computed 8 tokens at a time as a single
128x128x128 fp32 matmul whose cross-token blocks are pushed to -1024 in
PSUM by a rank-8 bf16 mask matmul; exp() then zeroes them exactly, so the
block-diagonal softmax needs no masking pass on DVE.
"""
import sys
sys.path.insert(0, "/opt/trn_rl_repo")
import numpy as np
import concourse.bass as bass
import concourse.mybir as mybir
import concourse.bacc as bacc
import concourse.tile as tile
from concourse.bass_utils import run_bass_kernel_spmd

B, S, D, H, HD = 4, 4096, 2048, 16, 128
NCORES = 8
T_FULL = B * S
F32, F32R, BF16 = mybir.dt.float32, mybir.dt.float32r, mybir.dt.bfloat16
KT = D // 128            # contraction tiles
SHIFT = 25.0             # constant softmax shift (softmax-invariant)
NEG = 1024.0             # additive mask magnitude for cross-token blocks
TA = 256                 # token chunk (phase A/B/C share this granularity)
Exp = mybir.ActivationFunctionType.Exp


def mask_consts():
    # u8[r,(t,h)] = 1 if t==r ; v8[r,(t',g)] = -NEG*(1 - (t'==r))
    u = np.zeros((8, 128), np.float32)
    for r in range(8):
        u[r, r * 16:(r + 1) * 16] = 1.0
    v = np.full((8, 128), -NEG, np.float32)
    for r in range(8):
        v[r, r * 16:(r + 1) * 16] = 0.0
    return u, v


def build(T, debug=False, repeat=1, trace_sim=False):
    import ml_dtypes
    TAe = min(TA, T)
    NCH = T // TAe           # chunks
    NBK = TAe // 8           # 8-token blocks per chunk
    nc = bacc.Bacc(None, target_bir_lowering=False)
    dt_in = lambda n, s: nc.dram_tensor(n, s, F32R, kind="ExternalInput")
    qT = dt_in("qT", [D, T]); kT = dt_in("kT", [D, T]); vT = dt_in("vT", [D, T])
    WqT = dt_in("WqT", [D, D]); WkT = dt_in("WkT", [D, D])
    WvT = dt_in("WvT", [D, D]); WoT = dt_in("WoT", [D, D])
    bqT = nc.dram_tensor("bqT", [128, H], F32, kind="ExternalInput")
    bkT = nc.dram_tensor("bkT", [128, H], F32, kind="ExternalInput")
    bvT = nc.dram_tensor("bvT", [128, H], F32, kind="ExternalInput")
    bo_row = nc.dram_tensor("bo_row", [1, D], F32R, kind="ExternalInput")
    ones_row = nc.dram_tensor("ones_row", [1, 128], F32R, kind="ExternalInput")
    out_d = nc.dram_tensor("out", [T, D], F32, kind="ExternalOutput")
    dbg = {}
    if debug:
        for n, shp in (("dQT", [128, T * H]), ("dKT", [128, T * H]),
                       ("dV", [128, T * H]), ("dATT", [D, T])):
            dbg[n] = nc.dram_tensor(n, shp, F32, kind="ExternalOutput")

    u8_np, v8_np = mask_consts()
    u8_d = nc.inline_tensor(u8_np.astype(ml_dtypes.bfloat16), "u8c")
    v8_d = nc.inline_tensor(v8_np.astype(ml_dtypes.bfloat16), "v8c")
    id_d = nc.inline_tensor(np.eye(128, dtype=np.float32), "id128").bitcast(F32R)

    with tile.TileContext(nc, trace_sim=trace_sim) as tc:
        with (
            tc.tile_pool(name="dram", bufs=1, space="DRAM") as dpool,
            tc.tile_pool(name="const", bufs=1) as cpool,
        ):
            # per-chunk spill tiles (fine-grained cross-phase deps)
            QT_ds = [dpool.tile([128, TAe * H], F32R, tag=f"QTd{i}", name=f"QTd{i}") for i in range(NCH)]
            KT_ds = [dpool.tile([128, TAe * H], F32R, tag=f"KTd{i}", name=f"KTd{i}") for i in range(NCH)]
            VT_ds = [dpool.tile([128, TAe * H], F32R, tag=f"VTd{i}", name=f"VTd{i}") for i in range(NCH)]
            ATT_ds = [dpool.tile([D, TAe], F32R, tag=f"ATTd{i}", name=f"ATTd{i}") for i in range(NCH)]

            u8 = cpool.tile([8, 128], BF16, tag="u8")
            v8 = cpool.tile([8, 128], BF16, tag="v8")
            ident = cpool.tile([128, 128], F32R, tag="ident")
            nc.sync.dma_start(u8[:], u8_d[:])
            nc.sync.dma_start(v8[:], v8_d[:])
            nc.sync.dma_start(ident[:], id_d[:])
            biasq = cpool.tile([128, H], F32, tag="bq")
            biask = cpool.tile([128, H], F32, tag="bk")
            biasv = cpool.tile([128, H], F32, tag="bvt")
            bor = cpool.tile([1, D], F32R, tag="bo")
            onesr = cpool.tile([1, 128], F32R, tag="ones")
            nc.sync.dma_start(biasq[:], bqT[:])
            nc.sync.dma_start(biask[:], bkT[:])
            nc.sync.dma_start(biasv[:], bvT[:])
            nc.sync.dma_start(bor[:], bo_row[:])
            nc.sync.dma_start(onesr[:], ones_row[:])
            shiftc = cpool.tile([128, 1], F32, tag="shiftc")
            nc.vector.memset(shiftc[:], -SHIFT)

            def _load_w(pool, win, tag):
                # separate quarter tiles -> first matmuls start after 1/4 load
                src = win.ap().rearrange("(it p) j -> p it j", p=128)
                parts = []
                for q in range(4):
                    wq = pool.tile([128, 4, D], F32R, tag=f"{tag}{q}", name=f"{tag}{q}")
                    nc.sync.dma_start(wq[:], src[:, q * 4:(q + 1) * 4, :])
                    parts.append(wq)
                return parts

            def _phases():
                # ---------------- Phase A: projections ----------------
                with (
                    tc.tile_pool(name="wt", bufs=1) as wpool,
                    tc.tile_pool(name="xs", bufs=2) as xpool,
                    tc.tile_pool(name="psA", bufs=8, space="PSUM") as psA,
                    tc.tile_pool(name="stA", bufs=1) as stA,
                ):
                    for xin, win, bias, spills in (
                        (qT, WqT, biasq, QT_ds),
                        (kT, WkT, biask, KT_ds),
                        (vT, WvT, biasv, VT_ds),
                    ):
                        xs0 = xpool.tile([128, KT, TAe], F32R, tag="xs", name="xs0")
                        nc.sync.dma_start(
                            xs0[:], xin[:, 0:TAe].rearrange("(it p) t -> p it t", p=128))
                        wt = _load_w(wpool, win, "wt")
                        for c in range(NCH):
                            if c == 0:
                                xs = xs0
                            else:
                                xs = xpool.tile([128, KT, TAe], F32R, tag="xs")
                                nc.sync.dma_start(
                                    xs[:], xin[:, c * TAe:(c + 1) * TAe].rearrange(
                                        "(it p) t -> p it t", p=128))
                            stg = stA.tile([128, TAe, H], F32R, tag="stA")
                            for jh in range(2):
                                pss = [psA.tile([128, TAe], F32, tag="psA",
                                                name=f"psA{jh}_{j}") for j in range(8)]
                                for q in range(4):
                                    for jl in range(8):
                                        jt = jh * 8 + jl
                                        for kl in range(4):
                                            ki = q * 4 + kl
                                            nc.tensor.matmul(
                                                pss[jl][:], wt[q][:, kl, jt * 128:(jt + 1) * 128],
                                                xs[:, ki, :], start=(ki == 0), stop=(ki == KT - 1))
                                for jl in range(8):
                                    jt = jh * 8 + jl
                                    nc.any.tensor_scalar_add(stg[:, :, jt], pss[jl][:],
                                                             bias[:, jt:jt + 1])
                            nc.sync.dma_start(
                                spills[c][:], stg[:].rearrange("p t h -> p (t h)"))

                # ---------------- Phase B (with Wo q0 prefetch) + C ----------------
                with tc.tile_pool(name="wo", bufs=1) as wopool:
                  wsrc = WoT.ap().rearrange("(h p) j -> p h j", p=128)
                  wo0 = wopool.tile([128, 4, D], F32R, tag="wo0", name="wo0")
                  nc.sync.dma_start(wo0[:], wsrc[:, 0:4, :])
                  with (
                      tc.tile_pool(name="qk", bufs=2) as qkpool,
                      tc.tile_pool(name="vb", bufs=2) as vpool,
                      tc.tile_pool(name="attc", bufs=2) as apool,
                      tc.tile_pool(name="eb", bufs=6) as epool,
                      tc.tile_pool(name="zb", bufs=8) as zpool,
                      tc.tile_pool(name="psS", bufs=2, space="PSUM") as psS,
                      tc.tile_pool(name="psT", bufs=2, space="PSUM") as psT,
                      tc.tile_pool(name="psV", bufs=2, space="PSUM") as psV,
                      tc.tile_pool(name="psA2", bufs=2, space="PSUM") as psA2,
                  ):
                      for c in range(NCH):
                          t0c = c * TAe
                          QTs = qkpool.tile([128, TAe, H], F32R, tag="QTs")
                          KTs = qkpool.tile([128, TAe, H], F32R, tag="KTs")
                          nc.gpsimd.dma_start(
                              QTs[:], QT_ds[c][:].rearrange("p (t h) -> p t h", h=H))
                          nc.gpsimd.dma_start(
                              KTs[:], KT_ds[c][:].rearrange("p (t h) -> p t h", h=H))
                          VTs = vpool.tile([128, TAe, H], F32R, tag="VTs")
                          nc.sync.dma_start(
                              VTs[:], VT_ds[c][:].rearrange("p (t h) -> p t h", h=H))
                          ATTc = apool.tile([128, H, TAe], F32R, tag="ATTc")
                          for bk in range(NBK):
                              sl = slice(bk * 8, (bk + 1) * 8)
                              w0 = (bk // 2) * 2            # even-aligned 2-block window
                              off = (bk % 2) * 128          # valid column offset
                              slw = slice(w0 * 8, (w0 + 2) * 8)
                              ps_b = psS.tile([128, 256], F32, tag="ps_s")
                              nc.tensor.matmul(
                                  ps_b[:],
                                  QTs[:, sl, :].rearrange("p t h -> p (t h)"),
                                  KTs[:, slw, :].rearrange("p t h -> p (t h)"),
                                  start=True, stop=False, skip_group_check=True)
                              nc.tensor.matmul(ps_b[:, off:off + 128], u8[:], v8[:],
                                               start=False, stop=True, skip_group_check=True)
                              E = epool.tile([128, 128], F32, tag="E")
                              Z = zpool.tile([128, 1], F32, tag="Z")
                              nc.scalar.activation(E[:], ps_b[:, off:off + 128], Exp,
                                                   bias=shiftc[:], accum_out=Z[:])
                              R = zpool.tile([128, 1], F32, tag="R")
                              nc.vector.reciprocal(R[:], Z[:])
                              Wb = epool.tile([128, 128], F32R, tag="Wb")
                              nc.vector.tensor_scalar_mul(Wb[:], E[:], R[:])
                              ps_t = psT.tile([128, 128], F32R, tag="ps_t")
                              nc.tensor.transpose(ps_t[:], Wb[:], ident[:])
                              WTs = epool.tile([128, 128], F32R, tag="WTs")
                              nc.any.tensor_copy(WTs[:], ps_t[:])
                              ps_v = psV.tile([128, 128], F32R, tag="ps_v")
                              nc.tensor.transpose(
                                  ps_v[:], VTs[:, sl, :].rearrange("p t h -> p (t h)"),
                                  ident[:])
                              Vb = epool.tile([128, 128], F32R, tag="Vb")
                              nc.any.tensor_copy(Vb[:], ps_v[:])
                              ps_a = psA2.tile([128, 128], F32, tag="ps_a")
                              nc.tensor.matmul(ps_a[:], Vb[:], WTs[:],
                                               start=True, stop=True)
                              nc.any.tensor_copy(
                                  ATTc[:, :, bk * 8:(bk + 1) * 8].rearrange("p h t -> p t h"),
                                  ps_a[:].rearrange("p (t h) -> p t h", t=8))
                          nc.sync.dma_start(
                              ATT_ds[c][:].rearrange("(h p) t -> p h t", p=128), ATTc[:])

                  # ---------------- Phase C: output projection ----------------
                  with (
                      tc.tile_pool(name="wo", bufs=1) as wopool,
                      tc.tile_pool(name="ca", bufs=2) as capool,
                      tc.tile_pool(name="psC", bufs=8, space="PSUM") as psC,
                      tc.tile_pool(name="stC", bufs=4) as stC,
                  ):
                      ATTs0 = capool.tile([128, H, TAe], F32R, tag="ATTs", name="ATTs0")
                      nc.sync.dma_start(
                          ATTs0[:], ATT_ds[0][:].rearrange("(h p) t -> p h t", p=128))
                      wo = [wo0]
                      for q in range(1, 4):
                          wq = wopool.tile([128, 4, D], F32R, tag=f"wo{q}", name=f"wo{q}")
                          nc.sync.dma_start(wq[:], wsrc[:, q * 4:(q + 1) * 4, :])
                          wo.append(wq)
                      for cc in range(NCH):
                          if cc == 0:
                              ATTs = ATTs0
                          else:
                              ATTs = capool.tile([128, H, TAe], F32R, tag="ATTs")
                              nc.sync.dma_start(
                                  ATTs[:], ATT_ds[cc][:].rearrange("(h p) t -> p h t", p=128))
                          tjs = [(tt, jc) for tt in range(TAe // 128) for jc in range(D // 512)]
                          pss = [psC.tile([128, 512], F32, tag="psC", name=f"psC{i}")
                                 for i in range(len(tjs))]
                          for hq in range(4):
                              for i, (tt, jc) in enumerate(tjs):
                                  for hl in range(4):
                                      h = hq * 4 + hl
                                      nc.tensor.matmul(
                                          pss[i][:], ATTs[:, h, tt * 128:(tt + 1) * 128],
                                          wo[hq][:, hl, jc * 512:(jc + 1) * 512],
                                          start=(h == 0), stop=False)
                          for i, (tt, jc) in enumerate(tjs):
                              nc.tensor.matmul(pss[i][:], onesr[:], bor[:, jc * 512:(jc + 1) * 512],
                                               start=False, stop=True)
                              st = stC.tile([128, 512], F32, tag="stC")
                              nc.any.tensor_copy(st[:], pss[i][:])
                              nc.sync.dma_start(
                                  out_d[cc * TAe + tt * 128: cc * TAe + (tt + 1) * 128,
                                        jc * 512:(jc + 1) * 512], st[:])

            for _rep in range(repeat):
                _phases()

            if debug:
                with tc.tile_pool(name="dbgp", bufs=2) as dbgp:
                    def dump(name, srcs, width):
                        for i, srct in enumerate(srcs):
                            flat = srct[:].rearrange("p a b -> p (a b)") \
                                if len(srct.shape) == 3 else srct[:]
                            rows = flat.shape[0]
                            for r0 in range(0, rows, 128):
                                tcp = dbgp.tile([128, width], F32, tag="dbg")
                                nc.sync.dma_start(tcp[:], flat[r0:r0 + 128, :].bitcast(F32))
                                nc.sync.dma_start(
                                    dbg[name][r0:r0 + 128, i * width:(i + 1) * width], tcp[:])
                    dump("dQT", QT_ds, TAe * H)
                    dump("dKT", KT_ds, TAe * H)
                    dump("dV", VT_ds, TAe * H)
                    dump("dATT", ATT_ds, TAe)
    nc.compile()
    return nc


_cache = {}


def get_nc(T, repeat=1):
    key = (T, repeat)
    if key not in _cache:
        _cache[key] = build(T, repeat=repeat)
    return _cache[key]


def make_in_maps(q, k, v, Wq, bq, Wk, bk, Wv, bv, Wo, bo, ncores=NCORES, T=None):
    f = np.float32
    q = np.asarray(q, f).reshape(-1, D)
    k = np.asarray(k, f).reshape(-1, D)
    v = np.asarray(v, f).reshape(-1, D)
    if T is None:
        T = q.shape[0] // ncores
    WqT = np.ascontiguousarray(np.asarray(Wq, f).T)
    WkT = np.ascontiguousarray(np.asarray(Wk, f).T)
    WvT = np.ascontiguousarray(np.asarray(Wv, f).T)
    WoT = np.ascontiguousarray(np.asarray(Wo, f).T)
    bqT = np.ascontiguousarray(np.asarray(bq, f).reshape(H, 128).T)
    bkT = np.ascontiguousarray(np.asarray(bk, f).reshape(H, 128).T)
    bvTc = np.ascontiguousarray(np.asarray(bv, f).reshape(H, 128).T)
    bor = np.asarray(bo, f).reshape(1, D)
    maps = []
    for c in range(ncores):
        sl = slice(c * T, (c + 1) * T)
        maps.append({
            "qT": np.ascontiguousarray(q[sl].T),
            "kT": np.ascontiguousarray(k[sl].T),
            "vT": np.ascontiguousarray(v[sl].T),
            "WqT": WqT, "WkT": WkT, "WvT": WvT, "WoT": WoT,
            "bqT": bqT, "bkT": bkT, "bvT": bvTc, "bo_row": bor,
            "ones_row": np.ones((1, 128), f),
        })
    return maps, T


def kernel(q, k, v, Wq, bq, Wk, bk, Wv, bv, Wo, bo):
    maps, T = make_in_maps(q, k, v, Wq, bq, Wk, bk, Wv, bv, Wo, bo)
    nc = get_nc(T)
    res = run_bass_kernel_spmd(nc, maps, list(range(NCORES)))
    out = np.concatenate([np.asarray(r["out"]) for r in res.results], axis=0)
    return out.reshape(B, S, D).astype(np.float32)


# revision 24
# speedup vs baseline: 51.4334x; 1.0768x over previous
"""Trainium2 Bass kernel for nn_MultiHeadAttention_67379446939752.

Per-token multi-head attention:
  Q = q @ Wq.T + bq ; K,V likewise        [B,S,D] -> [B,S,H,HD]
  score[t,h,g] = sum_d Q[t,h,d] K[t,g,d]  (per-token HxH gram, no seq mixing)
  attn[t] = softmax(score[t]) @ V[t]      -> [B,S,D]
  out = attn @ Wo.T + bo

Strategy: data-parallel over the 16384 tokens across 8 NeuronCores (2048
tokens/core).  All big matmuls run in float32r (full PE rate at N>=256,
~1e-4 relative error).  Host pre-transposes activations/weights so the
contraction dim lands on SBUF partitions with no on-device transposes.
The per-token 16x16 attention is computed 8 tokens at a time as a single
128x128x128 fp32 matmul whose cross-token blocks are pushed to -1024 in
PSUM by a rank-8 bf16 mask matmul; exp() then zeroes them exactly, so the
block-diagonal softmax needs no masking pass on DVE.
"""
import sys
sys.path.insert(0, "/opt/trn_rl_repo")
import numpy as np
import concourse.bass as bass
import concourse.mybir as mybir
import concourse.bacc as bacc
import concourse.tile as tile
from concourse.bass_utils import run_bass_kernel_spmd

B, S, D, H, HD = 4, 4096, 2048, 16, 128
NCORES = 8
T_FULL = B * S
F32, F32R, BF16 = mybir.dt.float32, mybir.dt.float32r, mybir.dt.bfloat16
KT = D // 128            # contraction tiles
SHIFT = 25.0             # constant softmax shift (softmax-invariant)
NEG = 1024.0             # additive mask magnitude for cross-token blocks
TA = 256                 # token chunk (phase A/B/C share this granularity)
Exp = mybir.ActivationFunctionType.Exp


def mask_consts():
    # u8[r,(t,h)] = 1 if t==r ; v8[r,(t',g)] = -NEG*(1 - (t'==r))
    u = np.zeros((8, 128), np.float32)
    for r in range(8):
        u[r, r * 16:(r + 1) * 16] = 1.0
    v = np.full((8, 128), -NEG, np.float32)
    for r in range(8):
        v[r, r * 16:(r + 1) * 16] = 0.0
    return u, v


def build(T, debug=False, repeat=1, trace_sim=False):
    import ml_dtypes
    TAe = min(TA, T)
    NCH = T // TAe           # chunks
    NBK = TAe // 8           # 8-token blocks per chunk
    nc = bacc.Bacc(None, target_bir_lowering=False)
    dt_in = lambda n, s: nc.dram_tensor(n, s, F32R, kind="ExternalInput")
    qT = dt_in("qT", [D, T]); kT = dt_in("kT", [D, T]); vT = dt_in("vT", [D, T])
    WqT = dt_in("WqT", [D, D]); WkT = dt_in("WkT", [D, D])
    WvT = dt_in("WvT", [D, D]); WoT = dt_in("WoT", [D, D])
    bqT = nc.dram_tensor("bqT", [128, H], F32, kind="ExternalInput")
    bkT = nc.dram_tensor("bkT", [128, H], F32, kind="ExternalInput")
    bvT = nc.dram_tensor("bvT", [128, H], F32, kind="ExternalInput")
    bo_row = nc.dram_tensor("bo_row", [1, D], F32R, kind="ExternalInput")
    ones_row = nc.dram_tensor("ones_row", [1, 128], F32R, kind="ExternalInput")
    out_d = nc.dram_tensor("out", [T, D], F32, kind="ExternalOutput")
    dbg = {}
    if debug:
        for n, shp in (("dQT", [128, T * H]), ("dKT", [128, T * H]),
                       ("dV", [128, T * H]), ("dATT", [D, T])):
            dbg[n] = nc.dram_tensor(n, shp, F32, kind="ExternalOutput")

    u8_np, v8_np = mask_consts()
    u8_d = nc.inline_tensor(u8_np.astype(ml_dtypes.bfloat16), "u8c")
    v8_d = nc.inline_tensor(v8_np.astype(ml_dtypes.bfloat16), "v8c")
    id_d = nc.inline_tensor(np.eye(128, dtype=ml_dtypes.bfloat16), "id128")

    with tile.TileContext(nc, trace_sim=trace_sim) as tc:
        with (
            tc.tile_pool(name="dram", bufs=1, space="DRAM") as dpool,
            tc.tile_pool(name="const", bufs=1) as cpool,
        ):
            # per-chunk spill tiles (fine-grained cross-phase deps)
            QT_ds = [dpool.tile([128, TAe * H], F32R, tag=f"QTd{i}", name=f"QTd{i}") for i in range(NCH)]
            KT_ds = [dpool.tile([128, TAe * H], F32R, tag=f"KTd{i}", name=f"KTd{i}") for i in range(NCH)]
            VT_ds = [dpool.tile([128, TAe * H], BF16, tag=f"VTd{i}", name=f"VTd{i}") for i in range(NCH)]
            ATT_ds = [dpool.tile([D, TAe], F32R, tag=f"ATTd{i}", name=f"ATTd{i}") for i in range(NCH)]

            u8 = cpool.tile([8, 128], BF16, tag="u8")
            v8 = cpool.tile([8, 128], BF16, tag="v8")
            ident = cpool.tile([128, 128], BF16, tag="ident")
            nc.sync.dma_start(u8[:], u8_d[:])
            nc.sync.dma_start(v8[:], v8_d[:])
            nc.sync.dma_start(ident[:], id_d[:])
            biasq = cpool.tile([128, H], F32, tag="bq")
            biask = cpool.tile([128, H], F32, tag="bk")
            biasv = cpool.tile([128, H], F32, tag="bvt")
            bor = cpool.tile([1, D], F32R, tag="bo")
            onesr = cpool.tile([1, 128], F32R, tag="ones")
            nc.sync.dma_start(biasq[:], bqT[:])
            nc.sync.dma_start(biask[:], bkT[:])
            nc.sync.dma_start(biasv[:], bvT[:])
            nc.sync.dma_start(bor[:], bo_row[:])
            nc.sync.dma_start(onesr[:], ones_row[:])
            shiftc = cpool.tile([128, 1], F32, tag="shiftc")
            nc.vector.memset(shiftc[:], -SHIFT)

            def _load_w(pool, win, tag):
                # separate quarter tiles -> first matmuls start after 1/4 load
                src = win.ap().rearrange("(it p) j -> p it j", p=128)
                parts = []
                for q in range(4):
                    wq = pool.tile([128, 4, D], F32R, tag=f"{tag}{q}", name=f"{tag}{q}")
                    nc.sync.dma_start(wq[:], src[:, q * 4:(q + 1) * 4, :])
                    parts.append(wq)
                return parts

            def _phases():
                # ---------------- Phase A: projections ----------------
                with (
                    tc.tile_pool(name="wt", bufs=1) as wpool,
                    tc.tile_pool(name="xs", bufs=2) as xpool,
                    tc.tile_pool(name="psA", bufs=8, space="PSUM") as psA,
                    tc.tile_pool(name="stA", bufs=1) as stA,
                ):
                    for xin, win, bias, spills, sdt in (
                        (qT, WqT, biasq, QT_ds, F32R),
                        (kT, WkT, biask, KT_ds, F32R),
                        (vT, WvT, biasv, VT_ds, BF16),
                    ):
                        xs0 = xpool.tile([128, KT, TAe], F32R, tag="xs", name="xs0")
                        nc.sync.dma_start(
                            xs0[:], xin[:, 0:TAe].rearrange("(it p) t -> p it t", p=128))
                        wt = _load_w(wpool, win, "wt")
                        for c in range(NCH):
                            if c == 0:
                                xs = xs0
                            else:
                                xs = xpool.tile([128, KT, TAe], F32R, tag="xs")
                                nc.sync.dma_start(
                                    xs[:], xin[:, c * TAe:(c + 1) * TAe].rearrange(
                                        "(it p) t -> p it t", p=128))
                            stg = stA.tile([128, TAe, H], sdt,
                                           tag="stA" if sdt == F32R else "stAv")
                            for jh in range(2):
                                pss = [psA.tile([128, TAe], F32, tag="psA",
                                                name=f"psA{jh}_{j}") for j in range(8)]
                                for q in range(4):
                                    for jl in range(8):
                                        jt = jh * 8 + jl
                                        for kl in range(4):
                                            ki = q * 4 + kl
                                            nc.tensor.matmul(
                                                pss[jl][:], wt[q][:, kl, jt * 128:(jt + 1) * 128],
                                                xs[:, ki, :], start=(ki == 0), stop=(ki == KT - 1))
                                for jl in range(8):
                                    jt = jh * 8 + jl
                                    nc.any.tensor_scalar_add(stg[:, :, jt], pss[jl][:],
                                                             bias[:, jt:jt + 1])
                            nc.sync.dma_start(
                                spills[c][:], stg[:].rearrange("p t h -> p (t h)"))

                # ---------------- Phase B (with Wo q0 prefetch) + C ----------------
                with tc.tile_pool(name="wo", bufs=1) as wopool:
                  wsrc = WoT.ap().rearrange("(h p) j -> p h j", p=128)
                  wo0 = wopool.tile([128, 4, D], F32R, tag="wo0", name="wo0")
                  nc.sync.dma_start(wo0[:], wsrc[:, 0:4, :])
                  with (
                      tc.tile_pool(name="qk", bufs=2) as qkpool,
                      tc.tile_pool(name="vb", bufs=2) as vpool,
                      tc.tile_pool(name="attc", bufs=2) as apool,
                      tc.tile_pool(name="eb", bufs=6) as epool,
                      tc.tile_pool(name="zb", bufs=8) as zpool,
                      tc.tile_pool(name="psS", bufs=2, space="PSUM") as psS,
                      tc.tile_pool(name="psT", bufs=2, space="PSUM") as psT,
                      tc.tile_pool(name="psV", bufs=2, space="PSUM") as psV,
                      tc.tile_pool(name="psA2", bufs=2, space="PSUM") as psA2,
                  ):
                      for c in range(NCH):
                          t0c = c * TAe
                          QTs = qkpool.tile([128, TAe, H], F32R, tag="QTs")
                          KTs = qkpool.tile([128, TAe, H], F32R, tag="KTs")
                          nc.gpsimd.dma_start(
                              QTs[:], QT_ds[c][:].rearrange("p (t h) -> p t h", h=H))
                          nc.gpsimd.dma_start(
                              KTs[:], KT_ds[c][:].rearrange("p (t h) -> p t h", h=H))
                          VTs = vpool.tile([128, TAe, H], BF16, tag="VTs")
                          nc.sync.dma_start(
                              VTs[:], VT_ds[c][:].rearrange("p (t h) -> p t h", h=H))
                          ATTc = apool.tile([128, H, TAe], F32R, tag="ATTc")
                          for bk in range(NBK):
                              sl = slice(bk * 8, (bk + 1) * 8)
                              w0 = (bk // 2) * 2            # even-aligned 2-block window
                              off = (bk % 2) * 128          # valid column offset
                              slw = slice(w0 * 8, (w0 + 2) * 8)
                              ps_b = psS.tile([128, 256], F32, tag="ps_s")
                              nc.tensor.matmul(
                                  ps_b[:],
                                  QTs[:, sl, :].rearrange("p t h -> p (t h)"),
                                  KTs[:, slw, :].rearrange("p t h -> p (t h)"),
                                  start=True, stop=False, skip_group_check=True)
                              nc.tensor.matmul(ps_b[:, off:off + 128], u8[:], v8[:],
                                               start=False, stop=True, skip_group_check=True)
                              E = epool.tile([128, 128], F32, tag="E")
                              Z = zpool.tile([128, 1], F32, tag="Z")
                              nc.scalar.activation(E[:], ps_b[:, off:off + 128], Exp,
                                                   bias=shiftc[:], accum_out=Z[:])
                              R = zpool.tile([128, 1], F32, tag="R")
                              nc.vector.reciprocal(R[:], Z[:])
                              Wb = epool.tile([128, 128], BF16, tag="Wb")
                              nc.vector.tensor_scalar_mul(Wb[:], E[:], R[:])
                              ps_t = psT.tile([128, 128], BF16, tag="ps_t")
                              nc.tensor.transpose(ps_t[:], Wb[:], ident[:])
                              WTs = epool.tile([128, 128], BF16, tag="WTs")
                              nc.any.tensor_copy(WTs[:], ps_t[:])
                              ps_v = psV.tile([128, 128], BF16, tag="ps_v")
                              nc.tensor.transpose(
                                  ps_v[:], VTs[:, sl, :].rearrange("p t h -> p (t h)"),
                                  ident[:])
                              Vb = epool.tile([128, 128], BF16, tag="Vb")
                              nc.any.tensor_copy(Vb[:], ps_v[:])
                              ps_a = psA2.tile([128, 128], F32, tag="ps_a")
                              nc.tensor.matmul(ps_a[:], Vb[:], WTs[:],
                                               start=True, stop=True)
                              nc.any.tensor_copy(
                                  ATTc[:, :, bk * 8:(bk + 1) * 8].rearrange("p h t -> p t h"),
                                  ps_a[:].rearrange("p (t h) -> p t h", t=8))
                          nc.sync.dma_start(
                              ATT_ds[c][:].rearrange("(h p) t -> p h t", p=128), ATTc[:])

                  # ---------------- Phase C: output projection ----------------
                  with (
                      tc.tile_pool(name="wo", bufs=1) as wopool,
                      tc.tile_pool(name="ca", bufs=2) as capool,
                      tc.tile_pool(name="psC", bufs=8, space="PSUM") as psC,
                      tc.tile_pool(name="stC", bufs=4) as stC,
                  ):
                      ATTs0 = capool.tile([128, H, TAe], F32R, tag="ATTs", name="ATTs0")
                      nc.sync.dma_start(
                          ATTs0[:], ATT_ds[0][:].rearrange("(h p) t -> p h t", p=128))
                      wo = [wo0]
                      for q in range(1, 4):
                          wq = wopool.tile([128, 4, D], F32R, tag=f"wo{q}", name=f"wo{q}")
                          nc.sync.dma_start(wq[:], wsrc[:, q * 4:(q + 1) * 4, :])
                          wo.append(wq)
                      for cc in range(NCH):
                          if cc == 0:
                              ATTs = ATTs0
                          else:
                              ATTs = capool.tile([128, H, TAe], F32R, tag="ATTs")
                              nc.sync.dma_start(
                                  ATTs[:], ATT_ds[cc][:].rearrange("(h p) t -> p h t", p=128))
                          tjs = [(tt, jc) for tt in range(TAe // 128) for jc in range(D // 512)]
                          pss = [psC.tile([128, 512], F32, tag="psC", name=f"psC{i}")
                                 for i in range(len(tjs))]
                          for hq in range(4):
                              for i, (tt, jc) in enumerate(tjs):
                                  for hl in range(4):
                                      h = hq * 4 + hl
                                      nc.tensor.matmul(
                                          pss[i][:], ATTs[:, h, tt * 128:(tt + 1) * 128],
                                          wo[hq][:, hl, jc * 512:(jc + 1) * 512],
                                          start=(h == 0), stop=False)
                          for i, (tt, jc) in enumerate(tjs):
                              nc.tensor.matmul(pss[i][:], onesr[:], bor[:, jc * 512:(jc + 1) * 512],
                                               start=False, stop=True)
                              st = stC.tile([128, 512], F32, tag="stC")
                              nc.any.tensor_copy(st[:], pss[i][:])
                              nc.sync.dma_start(
                                  out_d[cc * TAe + tt * 128: cc * TAe + (tt + 1) * 128,
                                        jc * 512:(jc + 1) * 512], st[:])

            for _rep in range(repeat):
                _phases()

            if debug:
                with tc.tile_pool(name="dbgp", bufs=2) as dbgp:
                    def dump(name, srcs, width):
                        for i, srct in enumerate(srcs):
                            flat = srct[:].rearrange("p a b -> p (a b)") \
                                if len(srct.shape) == 3 else srct[:]
                            rows = flat.shape[0]
                            for r0 in range(0, rows, 128):
                                tcp = dbgp.tile([128, width], F32, tag="dbg")
                                nc.sync.dma_start(tcp[:], flat[r0:r0 + 128, :].bitcast(F32))
                                nc.sync.dma_start(
                                    dbg[name][r0:r0 + 128, i * width:(i + 1) * width], tcp[:])
                    dump("dQT", QT_ds, TAe * H)
                    dump("dKT", KT_ds, TAe * H)
                    dump("dV", VT_ds, TAe * H)
                    dump("dATT", ATT_ds, TAe)
    nc.compile()
    return nc


_cache = {}


def get_nc(T, repeat=1):
    key = (T, repeat)
    if key not in _cache:
        _cache[key] = build(T, repeat=repeat)
    return _cache[key]


def make_in_maps(q, k, v, Wq, bq, Wk, bk, Wv, bv, Wo, bo, ncores=NCORES, T=None):
    f = np.float32
    q = np.asarray(q, f).reshape(-1, D)
    k = np.asarray(k, f).reshape(-1, D)
    v = np.asarray(v, f).reshape(-1, D)
    if T is None:
        T = q.shape[0] // ncores
    WqT = np.ascontiguousarray(np.asarray(Wq, f).T)
    WkT = np.ascontiguousarray(np.asarray(Wk, f).T)
    WvT = np.ascontiguousarray(np.asarray(Wv, f).T)
    WoT = np.ascontiguousarray(np.asarray(Wo, f).T)
    bqT = np.ascontiguousarray(np.asarray(bq, f).reshape(H, 128).T)
    bkT = np.ascontiguousarray(np.asarray(bk, f).reshape(H, 128).T)
    bvTc = np.ascontiguousarray(np.asarray(bv, f).reshape(H, 128).T)
    bor = np.asarray(bo, f).reshape(1, D)
    maps = []
    for c in range(ncores):
        sl = slice(c * T, (c + 1) * T)
        maps.append({
            "qT": np.ascontiguousarray(q[sl].T),
            "kT": np.ascontiguousarray(k[sl].T),
            "vT": np.ascontiguousarray(v[sl].T),
            "WqT": WqT, "WkT": WkT, "WvT": WvT, "WoT": WoT,
            "bqT": bqT, "bkT": bkT, "bvT": bvTc, "bo_row": bor,
            "ones_row": np.ones((1, 128), f),
        })
    return maps, T


def kernel(q, k, v, Wq, bq, Wk, bk, Wv, bv, Wo, bo):
    maps, T = make_in_maps(q, k, v, Wq, bq, Wk, bk, Wv, bv, Wo, bo)
    nc = get_nc(T)
    res = run_bass_kernel_spmd(nc, maps, list(range(NCORES)))
    out = np.concatenate([np.asarray(r["out"]) for r in res.results], axis=0)
    return out.reshape(B, S, D).astype(np.float32)
